# revision 25
# baseline (speedup 1.0000x reference)
"""DCT-compressed attention (nn_DCTAttentionIdeal) on 8 Trainium2 NeuronCores.

Math (per head): with P = Qd^T @ Qd (orthogonal projector, Qd orthonormal),
    out = P @ softmax(QK^T/8) @ P @ (V*mask)
Associativity lets us avoid the O(N^2 M) projection of the attention matrix:
    W   = P @ (V*mask)                  [N, D]   (cheap: 2x N*M*D)
    T   = exp(S/8) @ [1 | W]            [N, 1+D] (N^2*(D+1); the leading ones
                                                  column yields the softmax
                                                  denominator for free)
    out = P @ (T[:,1:] / T[:,0:1])      (cheap: 2x N*M*D)
This cuts TensorE work ~2.5x vs projecting atn into DCT space; the kernel is
then bound by ScalarE's exp throughput (~16.8M exps/core @ 128 lanes @1.2GHz).

Implementation notes:
  - Scores are computed TRANSPOSED (S^T[k,q] via lhsT=K^T) so exp output is
    directly the [k, q] layout the T-matmul wants as stationary.
  - Heads are processed in pairs stacked on partitions 0:64 / 64:128; the
    K=64 score matmuls for the two heads auto-pack into PE row-tiles
    (tile_position (0,0)/(64,0)) and run concurrently.
  - ALL matmul operands are bf16: f32r operands trigger the 4-pass fp32-HIGH
    PE path (4x slower) and poison FWL for neighboring bf16 weight loads.
    fp8 is not usable (W quantization cancels badly; exp overflows TRN
    fp8e4's +/-240 max). bf16 end-to-end measures ~4e-3 rel err.
  - T accumulation runs in four kb-passes (PSUM partials added into an SBUF
    f32 accumulator) so E tiles free progressively; the final pair's last
    pass covers a single kb so the epilogue after the last exp is small.
  - K/Q stream in via column-chunked DMAs so the first score matmul starts
    ~1us after launch instead of waiting for whole-tensor transfers.
  - mask (ones in this workload) is folded into K and V on the host.

Sharding: batch*heads (2*16=32) split 4-per-core across 8 cores; Q_dct
replicated; no cross-core communication.
"""

import numpy as np
import ml_dtypes

import concourse.tile as tile
from concourse import bacc, mybir
from concourse import bass_utils

F32 = mybir.dt.float32
BF16 = mybir.dt.bfloat16
NPBF16 = ml_dtypes.bfloat16
AF = mybir.ActivationFunctionType
ALU = mybir.AluOpType

B, H, N, D, M = 2, 16, 2048, 64, 256
NCORES = 8
HPC = (B * H) // NCORES   # heads per core = 4
NPAIR = HPC // 2          # head pairs per core = 2
NT = N // 128             # 16 k/q blocks
MT = M // 128             # 2
E_BUFS = 10               # bf16 E tiles in flight (pass sets + run-ahead)
# T-accumulation kb-pass boundaries per pair; the last pass is tiny so the
# epilogue after the final exp instruction is short.
PASSES = [
    [(0, 4), (4, 8), (8, 12), (12, 16)],
    [(0, 4), (4, 8), (8, 12), (12, 14), (14, 16)],
]


def _emit(tc, ctx, io):
    nc = tc.nc
    P = 128

    sh = ctx.enter_context(tc.tile_pool(name="shared", bufs=1))
    kq_pool = ctx.enter_context(tc.tile_pool(name="kq", bufs=2))
    v_pool = ctx.enter_context(tc.tile_pool(name="v", bufs=2))
    e_pool = ctx.enter_context(tc.tile_pool(name="exp", bufs=E_BUFS))
    w_pool = ctx.enter_context(tc.tile_pool(name="waug", bufs=2))
    vd_pool = ctx.enter_context(tc.tile_pool(name="vd", bufs=2))
    ta_pool = ctx.enter_context(tc.tile_pool(name="tacc", bufs=1))
    ts_pool = ctx.enter_context(tc.tile_pool(name="tsb", bufs=1))
    r1_pool = ctx.enter_context(tc.tile_pool(name="r1", bufs=2))
    ost_pool = ctx.enter_context(tc.tile_pool(name="ost", bufs=2))
    st_pool = ctx.enter_context(tc.tile_pool(name="stats", bufs=8))

    ps_s = ctx.enter_context(tc.tile_pool(name="ps_s", bufs=3, space="PSUM"))
    ps_u = ctx.enter_context(tc.tile_pool(name="ps_u", bufs=2, space="PSUM"))
    ps_t = ps_m = ps_u  # T-pass partials and small-matmul outputs share 2 banks

    state = [None] * NPAIR

    def prep_dma(p, chunked):
        st = state[p] = {"ex": {}}
        st["kt"] = kq_pool.tile([P, N], BF16, name="kt", tag="kt")
        st["qt"] = kq_pool.tile([P, N], BF16, name="qt", tag="qt")
        if chunked:  # first chunks of K and Q unblock the first matmuls fast
            nc.sync.dma_start(st["kt"][:, 0:512], io["KT2"][p, :, 0:512])
            for qc in range(4):
                nc.sync.dma_start(
                    st["qt"][:, qc * 512 : (qc + 1) * 512],
                    io["QT2"][p, :, qc * 512 : (qc + 1) * 512],
                )
            for kc in range(1, 4):
                nc.sync.dma_start(
                    st["kt"][:, kc * 512 : (kc + 1) * 512],
                    io["KT2"][p, :, kc * 512 : (kc + 1) * 512],
                )
        else:
            nc.sync.dma_start(st["kt"][:], io["KT2"][p])
            nc.sync.dma_start(st["qt"][:], io["QT2"][p])

    def v_dma(p):
        st = state[p]
        st["v"] = v_pool.tile([P, NT, P], BF16, name="v", tag="v")
        nc.sync.dma_start(st["v"][:], io["V2"][p])

    def prep_vd(p, mh, ts):
        # Vd2[m, dA|dB] = Qd @ V' (shared lhsT); emitted in half-chains so a
        # single filler never blocks the PE queue for long
        st = state[p]
        if mh == 0 and ts.start == 0:
            st["vd"] = vd_pool.tile([P, MT, P], BF16, name="vd", tag="vd")
        if ts.start == 0:
            st["vps"] = ps_m.tile([P, P], F32, name="mps", tag="u")
        for t in ts:
            nc.tensor.matmul(
                st["vps"][:],
                lhsT=qdtr[:, t, mh * P : (mh + 1) * P],
                rhs=st["v"][:, t, :],
                start=(t == 0),
                stop=(t == NT - 1),
            )
        if ts.stop == NT:
            nc.vector.tensor_copy(st["vd"][:, mh, :], st["vps"][:])

    def prep_w(p, nbs):
        # W2[n, 1+dA | 1+dB] = Qd^T @ Vd2, ones cols at 0 and D+1
        st = state[p]
        if 0 in nbs:
            st["wa"] = w_pool.tile([P, NT, 2 * (D + 1)], BF16, name="wa", tag="wa")
            nc.vector.memset(st["wa"][:, :, 0:1], 1.0)
            nc.vector.memset(st["wa"][:, :, D + 1 : D + 2], 1.0)
        wa = st["wa"]
        vd = st["vd"]
        for nb in nbs:
            if nb % 4 == 0:  # 4 sequential groups share one bank-sized tile
                st["wps4"] = ps_m.tile([P, 4, P], F32, name="mps", tag="u")
            wps4 = st["wps4"]
            for mh in range(MT):
                nc.tensor.matmul(
                    wps4[:, nb % 4, :],
                    lhsT=qdn[:, mh, nb * P : (nb + 1) * P],
                    rhs=vd[:, mh, :],
                    start=(mh == 0),
                    stop=(mh == MT - 1),
                )
            if nb % 4 == 3:
                nc.vector.tensor_copy(
                    wa[:, nb - 3 : nb + 1, 1 : D + 1], wps4[:, :, 0:D]
                )
                nc.vector.tensor_copy(
                    wa[:, nb - 3 : nb + 1, D + 2 : 2 * D + 2], wps4[:, :, D : 2 * D]
                )

    def s_exp(p, kb, fillers=()):
        # S^T[k, q] for both heads (row-packed K=64 matmuls), then exp -> bf16.
        # Filler work items are emitted between exp chunks so the PE queue
        # never puts a long burst ahead of the next chunk's score matmuls.
        st = state[p]
        ex = e_pool.tile([P, 2, N], BF16, name="ex", tag="ex")
        st["ex"][kb] = ex
        for qc in range(4):
            sps = ps_s.tile([P, 2, 512], F32, name="s", tag="s")
            for j in range(2):
                nc.tensor.matmul(
                    sps[:, j, :],
                    lhsT=st["kt"][64 * j : 64 * (j + 1), kb * P : (kb + 1) * P],
                    rhs=st["qt"][64 * j : 64 * (j + 1), qc * 512 : (qc + 1) * 512],
                    start=True,
                    stop=True,
                )
            nc.scalar.activation(
                ex[:, :, qc * 512 : (qc + 1) * 512],
                sps[:],
                AF.Exp,
                scale=0.125,
            )
            if qc < len(fillers):
                item = fillers[qc]
                item[0](*item[1:])
        for item in fillers[4:]:
            item[0](*item[1:])

    def t_begin(p):
        st = state[p]
        st["ta"] = ta_pool.tile([P, NT, 2, D + 1], F32, name="ta", tag="ta")
        st["ts"] = ts_pool.tile([P, NT, 2, D], BF16, name="ts", tag="ts")

    def t_pass(p, g, qp):
        # T[2qp:2qp+2, :] += sum_{kb in pass g} E^T-tile @ [1|W]  (both heads)
        st = state[p]
        k0, k1 = PASSES[p][g]
        ps = ps_t.tile([P, 2, 2, D + 1], F32, name="t", tag="u")
        for qi in range(2):
            qb = 2 * qp + qi
            for j in range(2):
                for kb in range(k0, k1):
                    nc.tensor.matmul(
                        ps[:, qi, j, :],
                        lhsT=st["ex"][kb][:, j, qb * P : (qb + 1) * P],
                        rhs=st["wa"][:, kb, j * (D + 1) : (j + 1) * (D + 1)],
                        start=(kb == k0),
                        stop=(kb == k1 - 1),
                    )
        ta = st["ta"]
        sl = ta[:, 2 * qp : 2 * qp + 2, :, :]
        if g == 0:
            nc.vector.tensor_copy(sl, ps[:])
        else:
            nc.vector.tensor_tensor(sl, sl, ps[:], op=ALU.add)
        if g == len(PASSES[p]) - 1:
            rec = st_pool.tile([P, 4], F32, name="rec", tag="rec")
            nc.vector.reciprocal(rec[:], ta[:, 2 * qp : 2 * qp + 2, :, 0:1])
            for qi in range(2):
                qb = 2 * qp + qi
                for j in range(2):
                    if p == 1 and j == 1:
                        # tail finalize: ScalarE is idle after the last exp
                        nc.scalar.activation(
                            st["ts"][:, qb, j, :],
                            ta[:, qb, j, 1 : D + 1],
                            AF.Copy,
                            scale=rec[:, 2 * qi + j : 2 * qi + j + 1],
                        )
                    else:
                        nc.vector.tensor_scalar_mul(
                            st["ts"][:, qb, j, :],
                            ta[:, qb, j, 1 : D + 1],
                            rec[:, 2 * qi + j : 2 * qi + j + 1],
                        )

    def tail_r1(p, mhs):
        st = state[p]
        if 0 in mhs:
            st["r1"] = r1_pool.tile([P, MT, P], BF16, name="r1", tag="r1")
            st["rps"] = ps_m.tile([P, MT, P], F32, name="mps", tag="u")
        for mh in mhs:
            for qb in range(NT):
                nc.tensor.matmul(
                    st["rps"][:, mh, :],
                    lhsT=qdtr[:, qb, mh * P : (mh + 1) * P],
                    rhs=st["ts"][:, qb, :, :],
                    start=(qb == 0),
                    stop=(qb == NT - 1),
                )
        if mhs[-1] == MT - 1:
            nc.vector.tensor_copy(st["r1"][:], st["rps"][:])

    def r1_begin(p):
        st = state[p]
        st["r1"] = r1_pool.tile([P, MT, P], BF16, name="r1", tag="r1")
        st["rps"] = [ps_m.tile([P, P], F32, name="mps", tag="u") for _ in range(MT)]

    def r1_qbs(p, qbs):
        st = state[p]
        for qb in qbs:
            for mh in range(MT):
                nc.tensor.matmul(
                    st["rps"][mh][:],
                    lhsT=qdtr[:, qb, mh * P : (mh + 1) * P],
                    rhs=st["ts"][:, qb, :, :],
                    start=(qb == 0),
                    stop=(qb == NT - 1),
                )
        if qbs[-1] == NT - 1:
            for mh in range(MT):
                nc.vector.tensor_copy(st["r1"][:, mh, :], st["rps"][mh][:])

    def tail_out(p, qbs):
        # 4 q-blocks share one SBUF staging tile and one 256KB DMA
        st = state[p]
        for qb in qbs:
            if qb % 4 == 0:
                st["ost"] = ost_pool.tile([P, 4, P], F32, name="ost", tag="ost")
                st["ops4"] = ps_m.tile([P, 4, P], F32, name="mps", tag="u")
            for mh in range(MT):
                nc.tensor.matmul(
                    st["ops4"][:, qb % 4, :],
                    lhsT=qdn[:, mh, qb * P : (qb + 1) * P],
                    rhs=st["r1"][:, mh, :],
                    start=(mh == 0),
                    stop=(mh == MT - 1),
                )
            if qb % 4 == 3:
                nc.vector.tensor_copy(st["ost"][:], st["ops4"][:])
                nc.sync.dma_start(io["out2"][p, qb // 4], st["ost"][:])

    # --- emission: 2 pair-windows, software-pipelined -------------------
    # warmup: tiny exp preloads the ACT table set during the initial DMAs
    wrm = st_pool.tile([P, 1], F32, name="wrm", tag="wrm")
    nc.vector.memset(wrm[:], 0.0)
    nc.scalar.activation(wrm[:], wrm[:], AF.Exp)

    prep_dma(0, chunked=True)
    qdtr = sh.tile([P, NT, M], BF16)    # Qd^T[n, m]: Vd lhsT + R1 lhsT
    nc.sync.dma_start(qdtr[:], io["QdT"].rearrange("(t p) m -> p t m", p=P))
    qdn = sh.tile([P, MT, N], BF16)     # Qd[m, n]: W lhsT + out lhsT
    nc.sync.dma_start(qdn[:], io["QdN"].rearrange("(c p) q -> p c q", p=P))
    v_dma(0)
    prep_dma(1, chunked=False)
    v_dma(1)

    s_exp(0, 0)
    s_exp(0, 1)
    t_begin(0)
    t_begin(1)
    # Per-slot PE work trailing each exp slot, kept under ~3.7us per slot
    # so the next slot's score matmuls are never queued behind a long burst.
    W0 = {
        2: [(prep_vd, 0, 0, range(0, 8)), (prep_vd, 0, 0, range(8, 16)),
            (prep_vd, 0, 1, range(0, 8)), (prep_vd, 0, 1, range(8, 16))],
        3: [(prep_w, 0, range(0, 2)), (prep_w, 0, range(2, 4)),
            (prep_w, 0, range(4, 6)), (prep_w, 0, range(6, 8))],
        4: [(prep_w, 0, range(8, 10)), (prep_w, 0, range(10, 12)),
            (prep_w, 0, range(12, 14)), (prep_w, 0, range(14, 16))],
        5: [(t_pass, 0, 0, 0), (t_pass, 0, 0, 1)],
        6: [(t_pass, 0, 0, 2), (t_pass, 0, 0, 3)],
        7: [(t_pass, 0, 0, 4), (t_pass, 0, 0, 5)],
        8: [(t_pass, 0, 0, 6), (t_pass, 0, 0, 7),
            (prep_vd, 1, 0, range(0, 8)), (prep_vd, 1, 0, range(8, 16))],
        9: [(t_pass, 0, 1, 0), (t_pass, 0, 1, 1),
            (prep_vd, 1, 1, range(0, 8)), (prep_vd, 1, 1, range(8, 16))],
        10: [(t_pass, 0, 1, 2), (t_pass, 0, 1, 3),
             (prep_w, 1, range(0, 2)), (prep_w, 1, range(2, 4))],
        11: [(t_pass, 0, 1, 4), (t_pass, 0, 1, 5),
             (prep_w, 1, range(4, 6)), (prep_w, 1, range(6, 8))],
        12: [(t_pass, 0, 1, 6), (t_pass, 0, 1, 7),
             (prep_w, 1, range(8, 10)), (prep_w, 1, range(10, 12))],
        13: [(t_pass, 0, 2, 0), (t_pass, 0, 2, 1),
             (prep_w, 1, range(12, 14)), (prep_w, 1, range(14, 16))],
        14: [(t_pass, 0, 2, 2), (t_pass, 0, 2, 3), (t_pass, 0, 2, 4)],
        15: [(t_pass, 0, 2, 5), (t_pass, 0, 2, 6), (t_pass, 0, 2, 7)],
    }
    for kb in range(2, NT):
        s_exp(0, kb, W0.get(kb, []))
    W1 = {
        0: [(t_pass, 0, 3, 0), (t_pass, 0, 3, 1)],
        1: [(t_pass, 0, 3, 2), (t_pass, 0, 3, 3)],
        2: [(t_pass, 0, 3, 4), (t_pass, 0, 3, 5)],
        3: [(t_pass, 0, 3, 6), (t_pass, 0, 3, 7)],
        4: [(t_pass, 1, 0, 0), (t_pass, 1, 0, 1), (tail_r1, 0, [0])],
        5: [(t_pass, 1, 0, 2), (t_pass, 1, 0, 3), (tail_r1, 0, [1])],
        6: [(t_pass, 1, 0, 4), (t_pass, 1, 0, 5), (tail_out, 0, range(0, 4))],
        7: [(t_pass, 1, 0, 6), (t_pass, 1, 0, 7), (tail_out, 0, range(4, 8))],
        8: [(t_pass, 1, 1, 0), (t_pass, 1, 1, 1), (tail_out, 0, range(8, 12))],
        9: [(t_pass, 1, 1, 2), (t_pass, 1, 1, 3), (tail_out, 0, range(12, 16))],
        10: [(t_pass, 1, 1, 4), (t_pass, 1, 1, 5)],
        11: [(t_pass, 1, 1, 6), (t_pass, 1, 1, 7)],
        12: [(t_pass, 1, 2, 0), (t_pass, 1, 2, 1), (t_pass, 1, 2, 2), (t_pass, 1, 2, 3)],
        13: [(t_pass, 1, 2, 4), (t_pass, 1, 2, 5), (t_pass, 1, 2, 6), (t_pass, 1, 2, 7)],
        14: [(t_pass, 1, 3, 0), (t_pass, 1, 3, 1), (t_pass, 1, 3, 2), (t_pass, 1, 3, 3),
             (t_pass, 1, 3, 4), (t_pass, 1, 3, 5), (t_pass, 1, 3, 6), (t_pass, 1, 3, 7)],
        15: [(t_pass, 1, 4, 0), (t_pass, 1, 4, 1), (t_pass, 1, 4, 2), (t_pass, 1, 4, 3),
             (t_pass, 1, 4, 4), (t_pass, 1, 4, 5), (t_pass, 1, 4, 6), (t_pass, 1, 4, 7)],
    }
    for kb in range(NT):
        s_exp(1, kb, W1.get(kb, []))
    # epilogue: only R1/out/DMA remain after the last exp
    tail_r1(1, [0])
    tail_r1(1, [1])
    tail_out(1, range(NT))


def build_nc():
    from contextlib import ExitStack

    nc = bacc.Bacc("TRN2", target_bir_lowering=False, debug=False)
    io = {
        "KT2": nc.dram_tensor("KT2", [NPAIR, 128, N], BF16, kind="ExternalInput").ap(),
        "QT2": nc.dram_tensor("QT2", [NPAIR, 128, N], BF16, kind="ExternalInput").ap(),
        "V2": nc.dram_tensor("V2", [NPAIR, 128, NT, 128], BF16, kind="ExternalInput").ap(),
        "QdT": nc.dram_tensor("QdT", [N, M], BF16, kind="ExternalInput").ap(),
        "QdN": nc.dram_tensor("QdN", [M, N], BF16, kind="ExternalInput").ap(),
        "out2": nc.dram_tensor(
            "out2", [NPAIR, 4, 128, 512], F32, kind="ExternalOutput"
        ).ap(),
    }
    with tile.TileContext(nc) as tc:
        with ExitStack() as ctx:
            _emit(tc, ctx, io)
    nc.compile()
    return nc


_NC = None


def _get_nc():
    global _NC
    if _NC is None:
        _NC = build_nc()
    return _NC


def make_in_maps(Q, K, V, mask, Q_dct):
    Q = np.asarray(Q, dtype=np.float32).reshape(B * H, N, D)
    K = np.asarray(K, dtype=np.float32).reshape(B * H, N, D)
    V = np.asarray(V, dtype=np.float32).reshape(B * H, N, D)
    mask = np.asarray(mask, dtype=np.float32)
    Q_dct = np.asarray(Q_dct, dtype=np.float32)

    # fold mask into K and V (host-side elementwise; mask is [B, N])
    mfull = np.repeat(mask, H, axis=0)[:, :, None]  # [B*H, N, 1]
    Km = K * mfull
    Vm = V * mfull

    QdT = np.ascontiguousarray(Q_dct.T).astype(NPBF16)
    QdN = np.ascontiguousarray(Q_dct).astype(NPBF16)

    in_maps = []
    for c in range(NCORES):
        KT2 = np.empty((NPAIR, 128, N), dtype=np.float32)
        QT2 = np.empty((NPAIR, 128, N), dtype=np.float32)
        V2 = np.empty((NPAIR, 128, NT, 128), dtype=np.float32)
        for p in range(NPAIR):
            for j in range(2):
                h = HPC * c + 2 * p + j
                KT2[p, 64 * j : 64 * (j + 1)] = Km[h].T
                QT2[p, 64 * j : 64 * (j + 1)] = Q[h].T
                # V'[n, d] -> [128(part), 16(nb), 64] at column offset 64*j
                V2[p, :, :, 64 * j : 64 * (j + 1)] = (
                    Vm[h].reshape(NT, 128, D).transpose(1, 0, 2)
                )
        in_maps.append(
            {
                "KT2": KT2.astype(NPBF16),
                "QT2": QT2.astype(NPBF16),
                "V2": V2.astype(NPBF16),
                "QdT": QdT,
                "QdN": QdN,
            }
        )
    return in_maps


def run_on_device(in_maps, **kwargs):
    nc = _get_nc()
    return bass_utils.run_bass_kernel_spmd(
        nc, in_maps, core_ids=list(range(NCORES)), **kwargs
    )


def kernel(Q, K, V, mask, Q_dct):
    in_maps = make_in_maps(Q, K, V, mask, Q_dct)
    res = run_on_device(in_maps)
    out = np.empty((B * H, N, D), dtype=np.float32)
    for c in range(NCORES):
        # [NPAIR, 4(qq), 128(r), 4(i), 2(j), 64(d)] -> per-head [N, D]
        o2 = res.results[c]["out2"].reshape(NPAIR, 4, 128, 4, 2, D)
        for p in range(NPAIR):
            for j in range(2):
                out[HPC * c + 2 * p + j] = (
                    o2[p, :, :, :, j, :].transpose(0, 2, 1, 3).reshape(N, D)
                )
    return out.reshape(B, H, N, D)
